# revision 6
# baseline (speedup 1.0000x reference)
"""Trainium2 Bass kernel for nn_DecSwitchedDeconv — PE-array-tiled per-sample convs.

Strategy (data-parallel, 32 samples/core, groups of 8):
  - conv1 runs as 8 concurrent (64x32) PE tiles (2 row-groups x 4 col-groups):
    tile k = sample k of the group, 9 taps x 3 chunks of N=362 accumulate into
    2 PSUM banks ([128,362] = 4 samples' 32-cout slices each).
  - conv2 runs as 8 concurrent (32x64) PE tiles: bank r holds samples (r, r+4)
    as 64-cout halves.
  - All routing/gather work happens on host: per-sample weight stacks are
    gathered by y_index, kernels flipped/transposed, z folded into W2 and
    b2*z folded into the residual input xq = x + b2*z. On-chip epilogue is a
    single tensor_tensor add (psum + xq). relu(x) = relu(xq - b2z) via
    tensor_scalar on gpsimd; conv1 bias+relu via scalar ACT from PSUM.
  - I/O in bf16 (x-residual and output upcast on host), span layout (34-wide
    padded rows) so every op is a contiguous [128, N] slab.
"""

import numpy as np

import concourse.bacc as bacc
import concourse.bass as bass
import concourse.mybir as mybir
import concourse.tile as tile
from concourse.bass_utils import run_bass_kernel_spmd

B, C, CSM, NB, HW = 256, 64, 32, 8, 32
M = 8                    # cores
BS = B // M              # 32 samples/core
NG = BS // 8             # 4 groups of 8 samples
WP = HW + 2              # 34
L = WP * WP              # 1156
NVAL = (HW - 1) * WP + HW  # 1086 span covering all valid outputs
BASE = WP + 1            # 35
NCH = 3
CH = NVAL // NCH         # 362
NBUF = 2

f32 = mybir.dt.float32
bf16 = mybir.dt.bfloat16


def _build_bass():
    nc = bacc.Bacc(target_bir_lowering=False, debug=False)
    # per-core inputs (host pre-gathered/packed, all static)
    xqg = nc.dram_tensor("xqg", [NG * 128, 4 * NVAL], bf16, kind="ExternalInput")
    wg = nc.dram_tensor("wg", [NG * 128, 2304], bf16, kind="ExternalInput")
    b1g = nc.dram_tensor("b1g", [128, 2 * NG], f32, kind="ExternalInput")
    b2zg = nc.dram_tensor("b2zg", [128, 4 * NG], f32, kind="ExternalInput")
    outd = nc.dram_tensor("out", [NG * 4 * 128, HW * WP], bf16, kind="ExternalOutput")

    add = mybir.AluOpType.add
    Relu = mybir.ActivationFunctionType.Relu

    with tile.TileContext(nc) as tc:
        # persistent padded tensors; borders zeroed once and never rewritten
        xpads, hpas, hpbs = [], [], []
        for i in range(NBUF):
            xp = nc.alloc_sbuf_tensor(f"xpad{i}", [128, 4 * L], bf16).ap()
            ha = nc.alloc_sbuf_tensor(f"hpa{i}", [128, L], bf16).ap()
            hb = nc.alloc_sbuf_tensor(f"hpb{i}", [128, L], bf16).ap()
            for k in range(4):
                nc.vector.memset(xp[:, k * L:k * L + BASE], 0.0)
                nc.vector.memset(xp[:, k * L + BASE + NVAL:(k + 1) * L], 0.0)
            nc.gpsimd.memset(ha[:, 0:BASE], 0.0)
            nc.gpsimd.memset(ha[:, BASE + NVAL:L], 0.0)
            nc.gpsimd.memset(hb[:, 0:BASE], 0.0)
            nc.gpsimd.memset(hb[:, BASE + NVAL:L], 0.0)
            xpads.append(xp)
            hpas.append(ha)
            hpbs.append(hb)
        warm_sb = nc.alloc_sbuf_tensor("warm_sb", [128, 640], bf16).ap()
        b1t = nc.alloc_sbuf_tensor("b1t", [128, 2 * NG], f32).ap()
        b2zt = nc.alloc_sbuf_tensor("b2zt", [128, 4 * NG], f32).ap()
        nc.sync.dma_start(b2zt, b2zg.ap())

        with (
            tc.tile_pool(name="io", bufs=3) as iop,
            tc.tile_pool(name="wp", bufs=3) as wpp,
            tc.tile_pool(name="osp", bufs=3) as ospp,
            tc.tile_pool(name="ps", bufs=8, space="PSUM") as psp,
        ):
            # HAM pre-warm: dummy full-array matmuls while first loads land
            wps = psp.tile([128, CH], f32, name="warm_ps", tag="ps")
            for i in range(28):
                nc.tensor.matmul(
                    wps[:, 0:256], lhsT=warm_sb[:, 512:640],
                    rhs=warm_sb[:, 0:256], start=True, stop=True)

            def emit_loads(g):
                xqall = iop.tile([128, 4 * NVAL], bf16, tag="xq", name=f"xq_{g}")
                wsb = wpp.tile([128, 2304], bf16, tag="w", name=f"w_{g}")
                nc.gpsimd.dma_start(wsb[:, :], wg.ap()[g * 128:(g + 1) * 128, :])
                if g == 0:
                    for k in range(4):
                        nc.sync.dma_start(
                            xqall[:, k * NVAL:(k + 1) * NVAL],
                            xqg.ap()[g * 128:(g + 1) * 128, k * NVAL:(k + 1) * NVAL])
                    nc.sync.dma_start(b1t, b1g.ap())
                else:
                    nc.sync.dma_start(xqall[:, 0:2 * NVAL],
                                      xqg.ap()[g * 128:(g + 1) * 128, 0:2 * NVAL])
                    nc.sync.dma_start(xqall[:, 2 * NVAL:4 * NVAL],
                                      xqg.ap()[g * 128:(g + 1) * 128, 2 * NVAL:4 * NVAL])
                return xqall, wsb

            def emit_relu(g, xqs, xpad):
                xp3 = xpad.rearrange("p (b h w) -> p b h w", b=4, w=WP)
                for k in range(4):
                    if g == 0:
                        nc.vector.tensor_scalar(
                            xpad[:, k * L + BASE:k * L + BASE + NVAL], xqs[k][:, :],
                            b2zt[:, 4 * g + k:4 * g + k + 1], 0.0,
                            op0=add, op1=mybir.AluOpType.max)
                    else:
                        nc.scalar.activation(
                            xpad[:, k * L + BASE:k * L + BASE + NVAL], xqs[k][:, :],
                            Relu, bias=b2zt[:, 4 * g + k:4 * g + k + 1])
                    nc.gpsimd.memset(xp3[:, k, 1:HW + 1, 0:WP:WP - 1], 0.0)

            def emit_conv1(g, w1sb, xpad, ha, hb):
                for c in range(NCH):
                    psa = psp.tile([128, CH], f32, name=f"ps1a_{g}_{c}", tag="ps")
                    psb = psp.tile([128, CH], f32, name=f"ps1b_{g}_{c}", tag="ps")
                    for t in range(9):
                        dy, dx = divmod(t, 3)
                        off = dy * WP + dx + c * CH
                        for k in range(8):
                            row = 64 * (k // 4)
                            col = 32 * (k % 4)
                            ps = psa if k < 4 else psb
                            nc.tensor.matmul(
                                ps[col:col + 32, :],
                                lhsT=w1sb[row:row + 64,
                                          (k % 4) * 288 + t * 32:(k % 4) * 288 + (t + 1) * 32],
                                rhs=xpad[row:row + 64,
                                         (k % 4) * L + off:(k % 4) * L + off + CH],
                                start=(t == 0), stop=(t == 8),
                                tile_position=(row, col),
                            )
                    nc.vector.tensor_scalar(
                        ha[:, BASE + c * CH:BASE + (c + 1) * CH], psa[:, :],
                        b1t[:, 2 * g:2 * g + 1], 0.0,
                        op0=add, op1=mybir.AluOpType.max)
                    nc.vector.tensor_scalar(
                        hb[:, BASE + c * CH:BASE + (c + 1) * CH], psb[:, :],
                        b1t[:, 2 * g + 1:2 * g + 2], 0.0,
                        op0=add, op1=mybir.AluOpType.max)
                ha3 = ha.rearrange("p (h w) -> p h w", w=WP)
                hb3 = hb.rearrange("p (h w) -> p h w", w=WP)
                nc.gpsimd.memset(ha3[:, 1:HW + 1, 0:WP:WP - 1], 0.0)
                nc.gpsimd.memset(hb3[:, 1:HW + 1, 0:WP:WP - 1], 0.0)

            def emit_conv2(g, w2sb, xqs, ha, hb):
                outs = []
                for r in range(4):
                    osp = ospp.tile([128, 32 * WP], bf16, tag=f"os{r}",
                                    name=f"os{r}_{g}")
                    outs.append(osp)
                chunks = ([(0, 512), (512, 512), (1024, 62)] if g == NG - 1
                          else [(0, CH), (CH, CH), (2 * CH, CH)])
                for c, (co, cn) in enumerate(chunks):
                    pss = [psp.tile([128, 512], f32, name=f"ps2_{g}_{c}_{r}",
                                    tag="ps")
                           for r in range(4)]
                    for t in range(9):
                        dy, dx = divmod(t, 3)
                        off = dy * WP + dx + co
                        for k in range(8):
                            r, h = divmod(k, 2)
                            src = ha if h == 0 else hb
                            nc.tensor.matmul(
                                pss[r][64 * h:64 * h + 64, 0:cn],
                                lhsT=w2sb[32 * r:32 * r + 32,
                                          h * 576 + t * 64:h * 576 + (t + 1) * 64],
                                rhs=src[32 * r:32 * r + 32, off:off + cn],
                                start=(t == 0), stop=(t == 8),
                                tile_position=(32 * r, 64 * h),
                            )
                    for r in range(4):
                        nc.vector.tensor_tensor(
                            outs[r][:, co:co + cn], pss[r][:, 0:cn],
                            xqs[r][:, co:co + cn], op=add)
                        eng = nc.sync if r % 2 == 0 else nc.gpsimd
                        eng.dma_start(
                            outd.ap()[(g * 4 + r) * 128:(g * 4 + r + 1) * 128,
                                      co:co + cn],
                            outs[r][:, co:co + cn])

            # pair-structured emission: conv1(2p), conv1(2p+1), conv2(2p),
            # conv2(2p+1) — halves PE tiling-mode switches and overlaps the
            # second group's relu/evac under same-mode matmul streams.
            pending = emit_loads(0)
            for p in range(NG // 2):
                ga, gb = 2 * p, 2 * p + 1
                sa, sb = [], []
                for g, st in ((ga, sa), (gb, sb)):
                    xqall, wsb = pending if g == ga else pending_b
                    st.extend([
                        xpads[g % NBUF], hpas[g % NBUF], hpbs[g % NBUF],
                        [xqall[:, k * NVAL:(k + 1) * NVAL] for k in range(4)],
                        wsb[:, 0:1152], wsb[:, 1152:2304],
                    ])
                    if g == ga:
                        emit_relu(ga, sa[3], sa[0])
                        pending_b = emit_loads(gb)
                xpad_a, ha_a, hb_a, xqs_a, w1_a, w2_a = sa
                xpad_b, ha_b, hb_b, xqs_b, w1_b, w2_b = sb
                emit_conv1(ga, w1_a, xpad_a, ha_a, hb_a)
                emit_relu(gb, xqs_b, xpad_b)
                emit_conv1(gb, w1_b, xpad_b, ha_b, hb_b)
                if gb + 1 < NG:
                    pending = emit_loads(gb + 1)
                emit_conv2(ga, w2_a, xqs_a, ha_a, hb_a)
                emit_conv2(gb, w2_b, xqs_b, ha_b, hb_b)


    nc.compile()
    return nc


import os as _os
if _os.environ.get("LDWOPT", "0") == "1":
    import concourse.bass_utils as _bu
    if not getattr(_bu, "_ldw_patched", False):
        _orig = _bu.run_command
        def _rc(argv, **kw):
            argv = ["--enable-ldw-opt=true" if a == "--enable-ldw-opt=false" else a
                    for a in argv]
            return _orig(argv, **kw)
        _bu.run_command = _rc
        _bu._ldw_patched = True

_NC = None


def _get_nc():
    global _NC
    if _NC is None:
        _NC = _build_bass()
    return _NC


def _host_prep(x, y_index, z, W1, b1, W2, b2):
    import ml_dtypes
    idx = np.asarray(y_index).reshape(B).astype(np.int64)
    # flipped-kernel stacks: w1t [NB, C, 9, CSM], w2t [NB, CSM, 9, C]
    w1t = np.ascontiguousarray(
        W1[:, :, :, ::-1, ::-1].transpose(0, 1, 3, 4, 2)).reshape(NB, C, 9, CSM)
    w2t = np.ascontiguousarray(
        W2[:, :, :, ::-1, ::-1].transpose(0, 1, 3, 4, 2)).reshape(NB, CSM, 9, C)
    w1s = w1t[idx]                                   # [B, 64, 9, 32] f32
    w2s = w2t[idx] * z[:, None, None, :]             # [B, 32, 9, 64] f32
    b2z = b2[idx] * z                                # [B, 64]
    b1s = b1[idx]                                    # [B, 32]

    # xq = x + b2z, span layout [B, 64, 1086] with zeros at junk cols
    xq = x + b2z[:, :, None, None]
    xqp = np.zeros((B, C, WP, WP), np.float32)
    xqp[:, :, 1:HW + 1, 1:HW + 1] = xq
    xq_span = xqp.reshape(B, C, L)[:, :, BASE:BASE + NVAL].copy()
    ji = np.array([i for i in range(NVAL) if (BASE + i) % WP in (0, WP - 1)])
    xq_span[:, :, ji] = -1e30
    xq_span = xq_span.astype(ml_dtypes.bfloat16)

    w1sb = w1s.astype(ml_dtypes.bfloat16)
    w2sb = w2s.astype(ml_dtypes.bfloat16)

    in_maps = []
    for cr in range(M):
        s0 = cr * BS
        # xqg rows: (g, k) pair tile = samples (s0+8g+k | s0+8g+4+k)
        xqg = np.empty((NG * 128, 4 * NVAL), ml_dtypes.bfloat16)
        wgh = np.zeros((NG * 128, 2304), ml_dtypes.bfloat16)
        b1h = np.zeros((128, 2 * NG), np.float32)
        b2zh = np.zeros((128, 4 * NG), np.float32)
        for g in range(NG):
            for k in range(4):
                sa, sb = s0 + 8 * g + k, s0 + 8 * g + 4 + k
                r0 = g * 128
                xqg[r0:r0 + 64, k * NVAL:(k + 1) * NVAL] = xq_span[sa]
                xqg[r0 + 64:r0 + 128, k * NVAL:(k + 1) * NVAL] = xq_span[sb]
                b2zh[0:64, 4 * g + k] = -b2z[sa]
                b2zh[64:128, 4 * g + k] = b2z[sb]
                # conv1 weights: tile k (cols k*288) top=sa, tile 8+k bottom=sb
                wgh[g * 128:g * 128 + 64,
                    k * 288:(k + 1) * 288] = w1s[sa].reshape(64, 288)
                wgh[g * 128 + 64:(g + 1) * 128,
                    k * 288:(k + 1) * 288] = w1s[sb].reshape(64, 288)
                # conv1 bias: bank a (cols 2g) = samples sa at 32*k..; bank b = sb
                b1h[32 * k:32 * (k + 1), 2 * g] = b1s[sa]
                b1h[32 * k:32 * (k + 1), 2 * g + 1] = b1s[sb]
                # conv2 weights: tile (32k, 64h): h=0 -> sa, h=1 -> sb
                wgh[g * 128 + 32 * k:g * 128 + 32 * (k + 1),
                    1152:1728] = w2sb[sa].reshape(32, 576)
                wgh[g * 128 + 32 * k:g * 128 + 32 * (k + 1),
                    1728:2304] = w2sb[sb].reshape(32, 576)
        in_maps.append(dict(xqg=xqg, wg=wgh, b1g=b1h, b2zg=b2zh))
    return in_maps


def kernel(x, y_index, y_hard, z, W1, b1, W2, b2, _trace=False):
    x = np.asarray(x, dtype=np.float32)
    z = np.asarray(z, dtype=np.float32)
    W1 = np.asarray(W1, dtype=np.float32)
    b1 = np.asarray(b1, dtype=np.float32)
    W2 = np.asarray(W2, dtype=np.float32)
    b2 = np.asarray(b2, dtype=np.float32)

    nc = _get_nc()
    in_maps = _host_prep(x, y_index, z, W1, b1, W2, b2)
    res = run_bass_kernel_spmd(nc, in_maps, core_ids=list(range(M)), trace=_trace)
    out = np.empty((B, C, HW, HW), np.float32)
    for cr in range(M):
        o = np.asarray(res.results[cr]["out"], dtype=np.float32)
        o = o.reshape(NG, 4, 2, C, HW, WP)[..., 0:HW]  # strip junk cols
        for g in range(NG):
            for k in range(4):
                out[cr * BS + 8 * g + k] = o[g, k, 0]
                out[cr * BS + 8 * g + 4 + k] = o[g, k, 1]
    if _trace:
        kernel._last_results = res
    return out


# revision 7
# speedup vs baseline: 1.1506x; 1.1506x over previous
"""Trainium2 Bass kernel for nn_DecSwitchedDeconv — PE-array-tiled per-sample convs.

Strategy (data-parallel, 32 samples/core, groups of 8):
  - conv1 runs as 8 concurrent (64x32) PE tiles (2 row-groups x 4 col-groups):
    tile k = sample k of the group, 9 taps x 3 chunks of N=362 accumulate into
    2 PSUM banks ([128,362] = 4 samples' 32-cout slices each).
  - conv2 runs as 8 concurrent (32x64) PE tiles: bank r holds samples (r, r+4)
    as 64-cout halves.
  - All routing/gather work happens on host: per-sample weight stacks are
    gathered by y_index, kernels flipped/transposed, z folded into W2 and
    b2*z folded into the residual input xq = x + b2*z; junk pad columns are
    painted -1e30 in xq so relu writes exact zeros there. On-chip epilogue is
    a single vector tensor_tensor add (psum + xq); conv1 bias+relu evac via
    vector tensor_scalar (keeps PSUM-release order sane on the vector queue);
    relu(x) via scalar ACT bias (vector for group 0 to dodge ACT-table load).
  - Pair-structured emission conv1(2p), conv1(2p+1), conv2(2p), conv2(2p+1)
    halves PE tiling-mode switches; one shared PSUM pool (bufs=8) rotates all
    banks; 28 dummy matmuls pre-warm the PE clock gate; loads prefetch ahead
    on sync/gpsimd queues, stores split across both; last group's conv2 uses
    ragged [512,512,62] chunks to shrink the final evac tail.
  - I/O in bf16 (x-residual and output upcast on host), span layout (34-wide
    padded rows) so every op is a contiguous [128, N] slab.
"""

import numpy as np

import concourse.bacc as bacc
import concourse.bass as bass
import concourse.mybir as mybir
import concourse.tile as tile
from concourse.bass_utils import run_bass_kernel_spmd

B, C, CSM, NB, HW = 256, 64, 32, 8, 32
M = 8                    # cores
BS = B // M              # 32 samples/core
NG = BS // 8             # 4 groups of 8 samples
WP = HW + 2              # 34
L = WP * WP              # 1156
NVAL = (HW - 1) * WP + HW  # 1086 span covering all valid outputs
BASE = WP + 1            # 35
NCH = 3
CH = NVAL // NCH         # 362
NBUF = 2

f32 = mybir.dt.float32
bf16 = mybir.dt.bfloat16


def _build_bass():
    nc = bacc.Bacc(target_bir_lowering=False, debug=False)
    # per-core inputs (host pre-gathered/packed, all static)
    xqg = nc.dram_tensor("xqg", [NG * 128, 4 * NVAL], bf16, kind="ExternalInput")
    wg = nc.dram_tensor("wg", [NG * 128, 2304], bf16, kind="ExternalInput")
    b1g = nc.dram_tensor("b1g", [128, 2 * NG], f32, kind="ExternalInput")
    b2zg = nc.dram_tensor("b2zg", [128, 4 * NG], f32, kind="ExternalInput")
    outd = nc.dram_tensor("out", [NG * 4 * 128, HW * WP], bf16, kind="ExternalOutput")

    add = mybir.AluOpType.add
    Relu = mybir.ActivationFunctionType.Relu

    with tile.TileContext(nc) as tc:
        # persistent padded tensors; borders zeroed once and never rewritten
        xpads, hpas, hpbs = [], [], []
        for i in range(NBUF):
            xp = nc.alloc_sbuf_tensor(f"xpad{i}", [128, 4 * L], bf16).ap()
            ha = nc.alloc_sbuf_tensor(f"hpa{i}", [128, L], bf16).ap()
            hb = nc.alloc_sbuf_tensor(f"hpb{i}", [128, L], bf16).ap()
            for k in range(4):
                nc.vector.memset(xp[:, k * L:k * L + BASE], 0.0)
                nc.vector.memset(xp[:, k * L + BASE + NVAL:(k + 1) * L], 0.0)
            nc.gpsimd.memset(ha[:, 0:BASE], 0.0)
            nc.gpsimd.memset(ha[:, BASE + NVAL:L], 0.0)
            nc.gpsimd.memset(hb[:, 0:BASE], 0.0)
            nc.gpsimd.memset(hb[:, BASE + NVAL:L], 0.0)
            xpads.append(xp)
            hpas.append(ha)
            hpbs.append(hb)
        warm_sb = nc.alloc_sbuf_tensor("warm_sb", [128, 640], bf16).ap()
        b1t = nc.alloc_sbuf_tensor("b1t", [128, 2 * NG], f32).ap()
        b2zt = nc.alloc_sbuf_tensor("b2zt", [128, 4 * NG], f32).ap()
        nc.sync.dma_start(b2zt, b2zg.ap())

        with (
            tc.tile_pool(name="io", bufs=3) as iop,
            tc.tile_pool(name="wp", bufs=3) as wpp,
            tc.tile_pool(name="osp", bufs=3) as ospp,
            tc.tile_pool(name="ps", bufs=8, space="PSUM") as psp,
        ):
            # HAM pre-warm: dummy full-array matmuls while first loads land
            wps = psp.tile([128, CH], f32, name="warm_ps", tag="ps")
            for i in range(28):
                nc.tensor.matmul(
                    wps[:, 0:256], lhsT=warm_sb[:, 512:640],
                    rhs=warm_sb[:, 0:256], start=True, stop=True)

            def emit_loads(g):
                xqall = iop.tile([128, 4 * NVAL], bf16, tag="xq", name=f"xq_{g}")
                wsb = wpp.tile([128, 2304], bf16, tag="w", name=f"w_{g}")
                nc.gpsimd.dma_start(wsb[:, :], wg.ap()[g * 128:(g + 1) * 128, :])
                if g == 0:
                    for k in range(4):
                        nc.sync.dma_start(
                            xqall[:, k * NVAL:(k + 1) * NVAL],
                            xqg.ap()[g * 128:(g + 1) * 128, k * NVAL:(k + 1) * NVAL])
                    nc.sync.dma_start(b1t, b1g.ap())
                else:
                    nc.sync.dma_start(xqall[:, 0:2 * NVAL],
                                      xqg.ap()[g * 128:(g + 1) * 128, 0:2 * NVAL])
                    nc.sync.dma_start(xqall[:, 2 * NVAL:4 * NVAL],
                                      xqg.ap()[g * 128:(g + 1) * 128, 2 * NVAL:4 * NVAL])
                return xqall, wsb

            def emit_relu(g, xqs, xpad):
                xp3 = xpad.rearrange("p (b h w) -> p b h w", b=4, w=WP)
                for k in range(4):
                    if g == 0:
                        nc.vector.tensor_scalar(
                            xpad[:, k * L + BASE:k * L + BASE + NVAL], xqs[k][:, :],
                            b2zt[:, 4 * g + k:4 * g + k + 1], 0.0,
                            op0=add, op1=mybir.AluOpType.max)
                    else:
                        nc.scalar.activation(
                            xpad[:, k * L + BASE:k * L + BASE + NVAL], xqs[k][:, :],
                            Relu, bias=b2zt[:, 4 * g + k:4 * g + k + 1])
                    nc.gpsimd.memset(xp3[:, k, 1:HW + 1, 0:WP:WP - 1], 0.0)

            def emit_conv1(g, w1sb, xpad, ha, hb):
                for c in range(NCH):
                    psa = psp.tile([128, CH], f32, name=f"ps1a_{g}_{c}", tag="ps")
                    psb = psp.tile([128, CH], f32, name=f"ps1b_{g}_{c}", tag="ps")
                    for t in range(9):
                        dy, dx = divmod(t, 3)
                        off = dy * WP + dx + c * CH
                        for k in range(8):
                            row = 64 * (k // 4)
                            col = 32 * (k % 4)
                            ps = psa if k < 4 else psb
                            nc.tensor.matmul(
                                ps[col:col + 32, :],
                                lhsT=w1sb[row:row + 64,
                                          (k % 4) * 288 + t * 32:(k % 4) * 288 + (t + 1) * 32],
                                rhs=xpad[row:row + 64,
                                         (k % 4) * L + off:(k % 4) * L + off + CH],
                                start=(t == 0), stop=(t == 8),
                                tile_position=(row, col),
                            )
                    nc.vector.tensor_scalar(
                        ha[:, BASE + c * CH:BASE + (c + 1) * CH], psa[:, :],
                        b1t[:, 2 * g:2 * g + 1], 0.0,
                        op0=add, op1=mybir.AluOpType.max)
                    nc.vector.tensor_scalar(
                        hb[:, BASE + c * CH:BASE + (c + 1) * CH], psb[:, :],
                        b1t[:, 2 * g + 1:2 * g + 2], 0.0,
                        op0=add, op1=mybir.AluOpType.max)
                ha3 = ha.rearrange("p (h w) -> p h w", w=WP)
                hb3 = hb.rearrange("p (h w) -> p h w", w=WP)
                nc.gpsimd.memset(ha3[:, 1:HW + 1, 0:WP:WP - 1], 0.0)
                nc.gpsimd.memset(hb3[:, 1:HW + 1, 0:WP:WP - 1], 0.0)

            def emit_conv2(g, w2sb, xqs, ha, hb):
                outs = []
                for r in range(4):
                    osp = ospp.tile([128, 32 * WP], bf16, tag=f"os{r}",
                                    name=f"os{r}_{g}")
                    outs.append(osp)
                chunks = ([(0, 512), (512, 512), (1024, 62)] if g == NG - 1
                          else [(0, CH), (CH, CH), (2 * CH, CH)])
                for c, (co, cn) in enumerate(chunks):
                    pss = [psp.tile([128, 512], f32, name=f"ps2_{g}_{c}_{r}",
                                    tag="ps")
                           for r in range(4)]
                    for t in range(9):
                        dy, dx = divmod(t, 3)
                        off = dy * WP + dx + co
                        for k in range(8):
                            r, h = divmod(k, 2)
                            src = ha if h == 0 else hb
                            nc.tensor.matmul(
                                pss[r][64 * h:64 * h + 64, 0:cn],
                                lhsT=w2sb[32 * r:32 * r + 32,
                                          h * 576 + t * 64:h * 576 + (t + 1) * 64],
                                rhs=src[32 * r:32 * r + 32, off:off + cn],
                                start=(t == 0), stop=(t == 8),
                                tile_position=(32 * r, 64 * h),
                            )
                    for r in range(4):
                        nc.vector.tensor_tensor(
                            outs[r][:, co:co + cn], pss[r][:, 0:cn],
                            xqs[r][:, co:co + cn], op=add)
                        eng = nc.sync if r % 2 == 0 else nc.gpsimd
                        eng.dma_start(
                            outd.ap()[(g * 4 + r) * 128:(g * 4 + r + 1) * 128,
                                      co:co + cn],
                            outs[r][:, co:co + cn])

            # pair-structured emission: conv1(2p), conv1(2p+1), conv2(2p),
            # conv2(2p+1) — halves PE tiling-mode switches and overlaps the
            # second group's relu/evac under same-mode matmul streams.
            pending = emit_loads(0)
            for p in range(NG // 2):
                ga, gb = 2 * p, 2 * p + 1
                sa, sb = [], []
                for g, st in ((ga, sa), (gb, sb)):
                    xqall, wsb = pending if g == ga else pending_b
                    st.extend([
                        xpads[g % NBUF], hpas[g % NBUF], hpbs[g % NBUF],
                        [xqall[:, k * NVAL:(k + 1) * NVAL] for k in range(4)],
                        wsb[:, 0:1152], wsb[:, 1152:2304],
                    ])
                    if g == ga:
                        emit_relu(ga, sa[3], sa[0])
                        pending_b = emit_loads(gb)
                xpad_a, ha_a, hb_a, xqs_a, w1_a, w2_a = sa
                xpad_b, ha_b, hb_b, xqs_b, w1_b, w2_b = sb
                emit_conv1(ga, w1_a, xpad_a, ha_a, hb_a)
                emit_relu(gb, xqs_b, xpad_b)
                emit_conv1(gb, w1_b, xpad_b, ha_b, hb_b)
                if gb + 1 < NG:
                    pending = emit_loads(gb + 1)
                emit_conv2(ga, w2_a, xqs_a, ha_a, hb_a)
                emit_conv2(gb, w2_b, xqs_b, ha_b, hb_b)


    nc.compile()
    return nc


import os as _os
if _os.environ.get("LDWOPT", "0") == "1":
    import concourse.bass_utils as _bu
    if not getattr(_bu, "_ldw_patched", False):
        _orig = _bu.run_command
        def _rc(argv, **kw):
            argv = ["--enable-ldw-opt=true" if a == "--enable-ldw-opt=false" else a
                    for a in argv]
            return _orig(argv, **kw)
        _bu.run_command = _rc
        _bu._ldw_patched = True

_NC = None


def _get_nc():
    global _NC
    if _NC is None:
        _NC = _build_bass()
    return _NC


def _host_prep(x, y_index, z, W1, b1, W2, b2):
    import ml_dtypes
    idx = np.asarray(y_index).reshape(B).astype(np.int64)
    # flipped-kernel stacks: w1t [NB, C, 9, CSM], w2t [NB, CSM, 9, C]
    w1t = np.ascontiguousarray(
        W1[:, :, :, ::-1, ::-1].transpose(0, 1, 3, 4, 2)).reshape(NB, C, 9, CSM)
    w2t = np.ascontiguousarray(
        W2[:, :, :, ::-1, ::-1].transpose(0, 1, 3, 4, 2)).reshape(NB, CSM, 9, C)
    w1s = w1t[idx]                                   # [B, 64, 9, 32] f32
    w2s = w2t[idx] * z[:, None, None, :]             # [B, 32, 9, 64] f32
    b2z = b2[idx] * z                                # [B, 64]
    b1s = b1[idx]                                    # [B, 32]

    # xq = x + b2z, span layout [B, 64, 1086] with zeros at junk cols
    xq = x + b2z[:, :, None, None]
    xqp = np.zeros((B, C, WP, WP), np.float32)
    xqp[:, :, 1:HW + 1, 1:HW + 1] = xq
    xq_span = xqp.reshape(B, C, L)[:, :, BASE:BASE + NVAL].copy()
    ji = np.array([i for i in range(NVAL) if (BASE + i) % WP in (0, WP - 1)])
    xq_span[:, :, ji] = -1e30
    xq_span = xq_span.astype(ml_dtypes.bfloat16)

    w1sb = w1s.astype(ml_dtypes.bfloat16)
    w2sb = w2s.astype(ml_dtypes.bfloat16)

    in_maps = []
    for cr in range(M):
        s0 = cr * BS
        # xqg rows: (g, k) pair tile = samples (s0+8g+k | s0+8g+4+k)
        xqg = np.empty((NG * 128, 4 * NVAL), ml_dtypes.bfloat16)
        wgh = np.zeros((NG * 128, 2304), ml_dtypes.bfloat16)
        b1h = np.zeros((128, 2 * NG), np.float32)
        b2zh = np.zeros((128, 4 * NG), np.float32)
        for g in range(NG):
            for k in range(4):
                sa, sb = s0 + 8 * g + k, s0 + 8 * g + 4 + k
                r0 = g * 128
                xqg[r0:r0 + 64, k * NVAL:(k + 1) * NVAL] = xq_span[sa]
                xqg[r0 + 64:r0 + 128, k * NVAL:(k + 1) * NVAL] = xq_span[sb]
                b2zh[0:64, 4 * g + k] = -b2z[sa]
                b2zh[64:128, 4 * g + k] = b2z[sb]
                # conv1 weights: tile k (cols k*288) top=sa, tile 8+k bottom=sb
                wgh[g * 128:g * 128 + 64,
                    k * 288:(k + 1) * 288] = w1s[sa].reshape(64, 288)
                wgh[g * 128 + 64:(g + 1) * 128,
                    k * 288:(k + 1) * 288] = w1s[sb].reshape(64, 288)
                # conv1 bias: bank a (cols 2g) = samples sa at 32*k..; bank b = sb
                b1h[32 * k:32 * (k + 1), 2 * g] = b1s[sa]
                b1h[32 * k:32 * (k + 1), 2 * g + 1] = b1s[sb]
                # conv2 weights: tile (32k, 64h): h=0 -> sa, h=1 -> sb
                wgh[g * 128 + 32 * k:g * 128 + 32 * (k + 1),
                    1152:1728] = w2sb[sa].reshape(32, 576)
                wgh[g * 128 + 32 * k:g * 128 + 32 * (k + 1),
                    1728:2304] = w2sb[sb].reshape(32, 576)
        in_maps.append(dict(xqg=xqg, wg=wgh, b1g=b1h, b2zg=b2zh))
    return in_maps


def kernel(x, y_index, y_hard, z, W1, b1, W2, b2, _trace=False):
    x = np.asarray(x, dtype=np.float32)
    z = np.asarray(z, dtype=np.float32)
    W1 = np.asarray(W1, dtype=np.float32)
    b1 = np.asarray(b1, dtype=np.float32)
    W2 = np.asarray(W2, dtype=np.float32)
    b2 = np.asarray(b2, dtype=np.float32)

    nc = _get_nc()
    in_maps = _host_prep(x, y_index, z, W1, b1, W2, b2)
    res = run_bass_kernel_spmd(nc, in_maps, core_ids=list(range(M)), trace=_trace)
    out = np.empty((B, C, HW, HW), np.float32)
    for cr in range(M):
        o = np.asarray(res.results[cr]["out"], dtype=np.float32)
        o = o.reshape(NG, 4, 2, C, HW, WP)[..., 0:HW]  # strip junk cols
        for g in range(NG):
            for k in range(4):
                out[cr * BS + 8 * g + k] = o[g, k, 0]
                out[cr * BS + 8 * g + 4 + k] = o[g, k, 1]
    if _trace:
        kernel._last_results = res
    return out


# revision 8
# speedup vs baseline: 1.1702x; 1.0171x over previous
"""Trainium2 Bass kernel for nn_DecSwitchedDeconv — PE-array-tiled per-sample convs.

Strategy (data-parallel, 32 samples/core, groups of 8):
  - conv1 runs as 8 concurrent (64x32) PE tiles (2 row-groups x 4 col-groups):
    tile k = sample k of the group, 9 taps x 3 chunks of N=362 accumulate into
    2 PSUM banks ([128,362] = 4 samples' 32-cout slices each).
  - conv2 runs as 8 concurrent (32x64) PE tiles: bank r holds samples (r, r+4)
    as 64-cout halves.
  - All routing/gather work happens on host: per-sample weight stacks are
    gathered by y_index, kernels flipped/transposed, z folded into W2 and
    b2*z folded into the residual input xq = x + b2*z; junk pad columns are
    painted -1e30 in xq so relu writes exact zeros there. On-chip epilogue is
    a single vector tensor_tensor add (psum + xq); conv1 bias+relu evac via
    vector tensor_scalar (keeps PSUM-release order sane on the vector queue);
    relu(x) via scalar ACT bias (vector for group 0 to dodge ACT-table load).
  - Pair-structured emission conv1(2p), conv1(2p+1), conv2(2p), conv2(2p+1)
    halves PE tiling-mode switches; one shared PSUM pool (bufs=8) rotates all
    banks; 28 dummy matmuls pre-warm the PE clock gate; loads prefetch ahead
    on sync/gpsimd queues, stores split across both; last group's conv2 uses
    ragged [512,512,62] chunks to shrink the final evac tail.
  - I/O in bf16 (x-residual and output upcast on host), span layout (34-wide
    padded rows) so every op is a contiguous [128, N] slab.
"""

import numpy as np

import concourse.bacc as bacc
import concourse.bass as bass
import concourse.mybir as mybir
import concourse.tile as tile
from concourse.bass_utils import run_bass_kernel_spmd

B, C, CSM, NB, HW = 256, 64, 32, 8, 32
M = 8                    # cores
BS = B // M              # 32 samples/core
NG = BS // 8             # 4 groups of 8 samples
WP = HW + 2              # 34
L = WP * WP              # 1156
NVAL = (HW - 1) * WP + HW  # 1086 span covering all valid outputs
BASE = WP + 1            # 35
NCH = 3
CH = NVAL // NCH         # 362
NBUF = 2

f32 = mybir.dt.float32
bf16 = mybir.dt.bfloat16


def _build_bass():
    nc = bacc.Bacc(target_bir_lowering=False, debug=False)
    # per-core inputs (host pre-gathered/packed, all static)
    xqg = nc.dram_tensor("xqg", [NG * 128, 4 * 1024], bf16, kind="ExternalInput")
    wg = nc.dram_tensor("wg", [NG * 128, 2304], bf16, kind="ExternalInput")
    b1g = nc.dram_tensor("b1g", [128, 2 * NG], f32, kind="ExternalInput")
    b2zg = nc.dram_tensor("b2zg", [128, 4 * NG], f32, kind="ExternalInput")
    outd = nc.dram_tensor("out", [NG * 4 * 128, 1024], bf16, kind="ExternalOutput")

    add = mybir.AluOpType.add
    Relu = mybir.ActivationFunctionType.Relu

    with tile.TileContext(nc) as tc:
        # persistent padded tensors; borders zeroed once and never rewritten
        xpads, hpas, hpbs = [], [], []
        for i in range(NBUF):
            xp = nc.alloc_sbuf_tensor(f"xpad{i}", [128, 4 * L], bf16).ap()
            ha = nc.alloc_sbuf_tensor(f"hpa{i}", [128, L], bf16).ap()
            hb = nc.alloc_sbuf_tensor(f"hpb{i}", [128, L], bf16).ap()
            nc.vector.memset(xp[:, 0:2 * L], 0.0)
            nc.vector.memset(xp[:, 2 * L:4 * L], 0.0)
            nc.gpsimd.memset(ha, 0.0)
            nc.gpsimd.memset(hb, 0.0)
            xpads.append(xp)
            hpas.append(ha)
            hpbs.append(hb)
        warm_sb = nc.alloc_sbuf_tensor("warm_sb", [128, 640], bf16).ap()
        b1t = nc.alloc_sbuf_tensor("b1t", [128, 2 * NG], f32).ap()
        b2zt = nc.alloc_sbuf_tensor("b2zt", [128, 4 * NG], f32).ap()
        nc.sync.dma_start(b2zt, b2zg.ap())

        with (
            tc.tile_pool(name="io", bufs=3) as iop,
            tc.tile_pool(name="wp", bufs=3) as wpp,
            tc.tile_pool(name="osp", bufs=3) as ospp,
            tc.tile_pool(name="ps", bufs=8, space="PSUM") as psp,
        ):
            # HAM pre-warm: dummy full-array matmuls while first loads land
            wps = psp.tile([128, CH], f32, name="warm_ps", tag="ps")
            for i in range(28):
                nc.tensor.matmul(
                    wps[:, 0:256], lhsT=warm_sb[:, 512:640],
                    rhs=warm_sb[:, 0:256], start=True, stop=True)

            def emit_loads(g):
                xqall = iop.tile([128, 4 * 1024], bf16, tag="xq", name=f"xq_{g}")
                wsb = wpp.tile([128, 2304], bf16, tag="w", name=f"w_{g}")
                nc.gpsimd.dma_start(wsb[:, :], wg.ap()[g * 128:(g + 1) * 128, :])
                if g == 0:
                    for k in range(4):
                        nc.sync.dma_start(
                            xqall[:, k * 1024:(k + 1) * 1024],
                            xqg.ap()[g * 128:(g + 1) * 128, k * 1024:(k + 1) * 1024])
                    nc.sync.dma_start(b1t, b1g.ap())
                else:
                    nc.sync.dma_start(xqall[:, 0:2048],
                                      xqg.ap()[g * 128:(g + 1) * 128, 0:2048])
                    nc.sync.dma_start(xqall[:, 2048:4096],
                                      xqg.ap()[g * 128:(g + 1) * 128, 2048:4096])
                return xqall, wsb

            def emit_relu(g, xqs, xpad):
                xp3 = xpad.rearrange("p (b h w) -> p b h w", b=4, w=WP)
                for k in range(4):
                    dst = xp3[:, k, 1:HW + 1, 1:HW + 1]
                    src = xqs[k].rearrange("p (h w) -> p h w", w=HW)
                    if g == 0:
                        nc.vector.tensor_scalar(
                            dst, src, b2zt[:, 4 * g + k:4 * g + k + 1], 0.0,
                            op0=add, op1=mybir.AluOpType.max)
                    else:
                        nc.scalar.activation(
                            dst, src, Relu,
                            bias=b2zt[:, 4 * g + k:4 * g + k + 1])

            def emit_conv1(g, w1sb, xpad, ha, hb):
                xp4 = xpad.rearrange("p (b h w) -> p b h w", b=4, w=WP)
                ha3 = ha.rearrange("p (h w) -> p h w", w=WP)
                hb3 = hb.rearrange("p (h w) -> p h w", w=WP)
                for c in range(2):
                    r0 = 16 * c
                    psa = psp.tile([128, 512], f32, name=f"ps1a_{g}_{c}", tag="ps")
                    psb = psp.tile([128, 512], f32, name=f"ps1b_{g}_{c}", tag="ps")
                    for t in range(9):
                        dy, dx = divmod(t, 3)
                        for k in range(8):
                            row = 64 * (k // 4)
                            col = 32 * (k % 4)
                            ps = psa if k < 4 else psb
                            nc.tensor.matmul(
                                ps[col:col + 32, :],
                                lhsT=w1sb[row:row + 64,
                                          (k % 4) * 288 + t * 32:(k % 4) * 288 + (t + 1) * 32],
                                rhs=xp4[row:row + 64, k % 4,
                                        r0 + dy:r0 + dy + 16, dx:dx + HW],
                                start=(t == 0), stop=(t == 8),
                                tile_position=(row, col),
                            )
                    nc.vector.tensor_scalar(
                        ha3[:, r0 + 1:r0 + 17, 1:HW + 1],
                        psa.rearrange("p (h w) -> p h w", w=HW),
                        b1t[:, 2 * g:2 * g + 1], 0.0,
                        op0=add, op1=mybir.AluOpType.max)
                    nc.vector.tensor_scalar(
                        hb3[:, r0 + 1:r0 + 17, 1:HW + 1],
                        psb.rearrange("p (h w) -> p h w", w=HW),
                        b1t[:, 2 * g + 1:2 * g + 2], 0.0,
                        op0=add, op1=mybir.AluOpType.max)

            def emit_conv2(g, w2sb, xqs, ha, hb):
                ha3 = ha.rearrange("p (h w) -> p h w", w=WP)
                hb3 = hb.rearrange("p (h w) -> p h w", w=WP)
                outs = []
                for r in range(4):
                    osp = ospp.tile([128, 1024], bf16, tag=f"os{r}",
                                    name=f"os{r}_{g}")
                    outs.append(osp)
                for c in range(2):
                    r0 = 16 * c
                    pss = [psp.tile([128, 512], f32, name=f"ps2_{g}_{c}_{r}",
                                    tag="ps")
                           for r in range(4)]
                    for t in range(9):
                        dy, dx = divmod(t, 3)
                        for k in range(8):
                            r, h = divmod(k, 2)
                            src3 = ha3 if h == 0 else hb3
                            nc.tensor.matmul(
                                pss[r][64 * h:64 * h + 64, :],
                                lhsT=w2sb[32 * r:32 * r + 32,
                                          h * 576 + t * 64:h * 576 + (t + 1) * 64],
                                rhs=src3[32 * r:32 * r + 32,
                                         r0 + dy:r0 + dy + 16, dx:dx + HW],
                                start=(t == 0), stop=(t == 8),
                                tile_position=(32 * r, 64 * h),
                            )
                    for r in range(4):
                        nc.vector.tensor_tensor(
                            outs[r][:, c * 512:(c + 1) * 512], pss[r][:, :],
                            xqs[r][:, c * 512:(c + 1) * 512], op=add)
                        eng = nc.sync if r % 2 == 0 else nc.gpsimd
                        eng.dma_start(
                            outd.ap()[(g * 4 + r) * 128:(g * 4 + r + 1) * 128,
                                      c * 512:(c + 1) * 512],
                            outs[r][:, c * 512:(c + 1) * 512])

            # pair-structured emission: conv1(2p), conv1(2p+1), conv2(2p),
            # conv2(2p+1) — halves PE tiling-mode switches and overlaps the
            # second group's relu/evac under same-mode matmul streams.
            pending = emit_loads(0)
            for p in range(NG // 2):
                ga, gb = 2 * p, 2 * p + 1
                sa, sb = [], []
                for g, st in ((ga, sa), (gb, sb)):
                    xqall, wsb = pending if g == ga else pending_b
                    st.extend([
                        xpads[g % NBUF], hpas[g % NBUF], hpbs[g % NBUF],
                        [xqall[:, k * 1024:(k + 1) * 1024] for k in range(4)],
                        wsb[:, 0:1152], wsb[:, 1152:2304],
                    ])
                    if g == ga:
                        emit_relu(ga, sa[3], sa[0])
                        pending_b = emit_loads(gb)
                xpad_a, ha_a, hb_a, xqs_a, w1_a, w2_a = sa
                xpad_b, ha_b, hb_b, xqs_b, w1_b, w2_b = sb
                emit_conv1(ga, w1_a, xpad_a, ha_a, hb_a)
                emit_relu(gb, xqs_b, xpad_b)
                emit_conv1(gb, w1_b, xpad_b, ha_b, hb_b)
                if gb + 1 < NG:
                    pending = emit_loads(gb + 1)
                emit_conv2(ga, w2_a, xqs_a, ha_a, hb_a)
                emit_conv2(gb, w2_b, xqs_b, ha_b, hb_b)


    nc.compile()
    return nc


import os as _os
if _os.environ.get("LDWOPT", "0") == "1":
    import concourse.bass_utils as _bu
    if not getattr(_bu, "_ldw_patched", False):
        _orig = _bu.run_command
        def _rc(argv, **kw):
            argv = ["--enable-ldw-opt=true" if a == "--enable-ldw-opt=false" else a
                    for a in argv]
            return _orig(argv, **kw)
        _bu.run_command = _rc
        _bu._ldw_patched = True

_NC = None


def _get_nc():
    global _NC
    if _NC is None:
        _NC = _build_bass()
    return _NC


def _host_prep(x, y_index, z, W1, b1, W2, b2):
    import ml_dtypes
    idx = np.asarray(y_index).reshape(B).astype(np.int64)
    # flipped-kernel stacks: w1t [NB, C, 9, CSM], w2t [NB, CSM, 9, C]
    w1t = np.ascontiguousarray(
        W1[:, :, :, ::-1, ::-1].transpose(0, 1, 3, 4, 2)).reshape(NB, C, 9, CSM)
    w2t = np.ascontiguousarray(
        W2[:, :, :, ::-1, ::-1].transpose(0, 1, 3, 4, 2)).reshape(NB, CSM, 9, C)
    w1s = w1t[idx]                                   # [B, 64, 9, 32] f32
    w2s = w2t[idx] * z[:, None, None, :]             # [B, 32, 9, 64] f32
    b2z = b2[idx] * z                                # [B, 64]
    b1s = b1[idx]                                    # [B, 32]

    # xq = x + b2z, dense [B, 64, 1024]
    xq = (x + b2z[:, :, None, None]).reshape(B, C, HW * HW)
    xq_span = xq.astype(ml_dtypes.bfloat16)

    w1sb = w1s.astype(ml_dtypes.bfloat16)
    w2sb = w2s.astype(ml_dtypes.bfloat16)

    in_maps = []
    for cr in range(M):
        s0 = cr * BS
        # xqg rows: (g, k) pair tile = samples (s0+8g+k | s0+8g+4+k)
        xqg = np.empty((NG * 128, 4 * 1024), ml_dtypes.bfloat16)
        wgh = np.zeros((NG * 128, 2304), ml_dtypes.bfloat16)
        b1h = np.zeros((128, 2 * NG), np.float32)
        b2zh = np.zeros((128, 4 * NG), np.float32)
        for g in range(NG):
            for k in range(4):
                sa, sb = s0 + 8 * g + k, s0 + 8 * g + 4 + k
                r0 = g * 128
                xqg[r0:r0 + 64, k * 1024:(k + 1) * 1024] = xq_span[sa]
                xqg[r0 + 64:r0 + 128, k * 1024:(k + 1) * 1024] = xq_span[sb]
                b2zh[0:64, 4 * g + k] = -b2z[sa]
                b2zh[64:128, 4 * g + k] = b2z[sb]
                # conv1 weights: tile k (cols k*288) top=sa, tile 8+k bottom=sb
                wgh[g * 128:g * 128 + 64,
                    k * 288:(k + 1) * 288] = w1s[sa].reshape(64, 288)
                wgh[g * 128 + 64:(g + 1) * 128,
                    k * 288:(k + 1) * 288] = w1s[sb].reshape(64, 288)
                # conv1 bias: bank a (cols 2g) = samples sa at 32*k..; bank b = sb
                b1h[32 * k:32 * (k + 1), 2 * g] = b1s[sa]
                b1h[32 * k:32 * (k + 1), 2 * g + 1] = b1s[sb]
                # conv2 weights: tile (32k, 64h): h=0 -> sa, h=1 -> sb
                wgh[g * 128 + 32 * k:g * 128 + 32 * (k + 1),
                    1152:1728] = w2sb[sa].reshape(32, 576)
                wgh[g * 128 + 32 * k:g * 128 + 32 * (k + 1),
                    1728:2304] = w2sb[sb].reshape(32, 576)
        in_maps.append(dict(xqg=xqg, wg=wgh, b1g=b1h, b2zg=b2zh))
    return in_maps


def kernel(x, y_index, y_hard, z, W1, b1, W2, b2, _trace=False):
    x = np.asarray(x, dtype=np.float32)
    z = np.asarray(z, dtype=np.float32)
    W1 = np.asarray(W1, dtype=np.float32)
    b1 = np.asarray(b1, dtype=np.float32)
    W2 = np.asarray(W2, dtype=np.float32)
    b2 = np.asarray(b2, dtype=np.float32)

    nc = _get_nc()
    in_maps = _host_prep(x, y_index, z, W1, b1, W2, b2)
    res = run_bass_kernel_spmd(nc, in_maps, core_ids=list(range(M)), trace=_trace)
    out = np.empty((B, C, HW, HW), np.float32)
    for cr in range(M):
        o = np.asarray(res.results[cr]["out"], dtype=np.float32)
        o = o.reshape(NG, 4, 2, C, HW, HW)
        for g in range(NG):
            for k in range(4):
                out[cr * BS + 8 * g + k] = o[g, k, 0]
                out[cr * BS + 8 * g + 4 + k] = o[g, k, 1]
    if _trace:
        kernel._last_results = res
    return out
